# revision 7
# baseline (speedup 1.0000x reference)
"""Trainium2 Bass kernel for the topk_masking L2P problem.

Computation (matches the reference nn.Module forward):
  q = l2norm(x_query); nK = l2norm(e_k)
  cos = einsum('blj,lkj->blk', q, nK)           # [B, nL, pool]
  idx = top_k(cos, 5).indices                   # [B, nL, 5]
  P_  = e_p[l, idx]                             # gather -> [nL, B, 25, emb]
  loss = sum(1 - topk_values) / B               # == nL*5 - sum(topk_vals)/B

Sharding over 8 cores: 4 layer-groups (3 layers each) x 2 batch-halves (64).

The dominant cost is materializing the gathered output (14.75MB/core HBM
write).  To avoid also re-reading e_p blocks from HBM (another 14.75MB),
most gather chunks are computed ON-CHIP as one-hot matmuls on the tensor
engine: e_p is decomposed once into three bf16 planes (hi/mid/lo, whose
f32 sum reconstructs e_p bit-exactly) and each output chunk is
Sel^T @ plane accumulated over the three planes in PSUM (exact), then
evacuated to SBUF and streamed out with large DMAs.  A few chunks go
through the SWDGE indirect-DMA gather instead to balance PE vs DMA time.
"""

import os

import numpy as np

import concourse.bacc as bacc
import concourse.bass as bass
import concourse.tile as tile
from concourse import mybir
from concourse.bass_utils import run_bass_kernel_spmd
from concourse.masks import make_identity

F32 = mybir.dt.float32
BF16 = mybir.dt.bfloat16
U32 = mybir.dt.uint32
I32 = mybir.dt.int32

B, NL, C = 128, 12, 768
POOL, PLEN, EMB = 30, 5, 768
TOPK = 5
NCORES = 8
LG = 3            # layers per core
BSH = 64          # batches per core
ROWS = LG * BSH   # 192 rows of (layer, batch)
NK = LG * POOL    # 90 keys per core
BLK = PLEN * EMB  # 3840 f32 per prompt block
JT = C // 128     # 6 contraction chunks
NB = 8            # psum-bank chunks per block-chunk (7x512 + 1x256)

LAST_EXEC_NS = None

# chunks (i, rg) routed through the DMA indirect gather instead of PE
N_DMA_CHUNKS = int(os.environ.get("KERNEL_DMA_CHUNKS", "2"))


def _build():
    nc = bacc.Bacc(
        "TRN2", target_bir_lowering=False, debug=False, num_devices=NCORES
    )
    xq = nc.dram_tensor("xq", [ROWS, C], F32, kind="ExternalInput")
    ek = nc.dram_tensor("ek", [NK, C], F32, kind="ExternalInput")
    ep = nc.dram_tensor("ep", [NK, BLK], F32, kind="ExternalInput")
    pout = nc.dram_tensor("pout", [ROWS * TOPK, BLK], F32, kind="ExternalOutput")
    lsum = nc.dram_tensor("lsum", [1, 1], F32, kind="ExternalOutput")

    AF = mybir.ActivationFunctionType

    # chunk list: (i, rowgroup) -> 128/64 output blocks each
    chunks = [(i, rg) for i in range(TOPK) for rg in (0, 1)]
    dma_chunks = set(chunks[:N_DMA_CHUNKS])

    with tile.TileContext(nc) as tc:
        with (
            tc.tile_pool(name="const", bufs=1) as constp,
            tc.tile_pool(name="work", bufs=1) as work,
            tc.tile_pool(name="planes", bufs=1) as planes,
            tc.tile_pool(name="selp", bufs=1) as selp,
            tc.tile_pool(name="stage", bufs=3) as stage,
        ):
            ident = constp.tile([128, 128], F32)
            make_identity(nc, ident[:])
            ones = constp.tile([128, 1], F32)
            nc.vector.memset(ones[:], 1.0)
            iota_i = constp.tile([128, 1], I32)
            nc.gpsimd.iota(iota_i[:], pattern=[[0, 1]], base=0,
                           channel_multiplier=1)
            iota_f = constp.tile([128, 1], F32)
            nc.vector.tensor_copy(out=iota_f[:], in_=iota_i[:])

            # ---- load inputs ----
            q0 = work.tile([128, C], F32)   # rows (l,b): l*64+b for l in {0,1}
            q1 = work.tile([64, C], F32)    # l = 2
            kt = work.tile([NK, C], F32)    # rows l*30+k
            nc.sync.dma_start(out=q0[:], in_=xq[0:128, :])
            nc.sync.dma_start(out=q1[:], in_=xq[128:192, :])
            nc.sync.dma_start(out=kt[:], in_=ek[:, :])

            # ---- e_p -> bf16 planes (hi+mid+lo sums back to f32 exactly) ----
            epf = work.tile([NK, BLK], F32)
            nc.sync.dma_start(out=epf[:], in_=ep[:, :])
            hi = planes.tile([NK, BLK], BF16)
            md = planes.tile([NK, BLK], BF16)
            lo = planes.tile([NK, BLK], BF16)
            res = work.tile([NK, BLK], F32)
            nc.vector.tensor_copy(out=hi[:], in_=epf[:])
            nc.vector.tensor_tensor(out=res[:], in0=epf[:], in1=hi[:],
                                    op=mybir.AluOpType.subtract)
            nc.vector.tensor_copy(out=md[:], in_=res[:])
            nc.vector.tensor_tensor(out=res[:], in0=res[:], in1=md[:],
                                    op=mybir.AluOpType.subtract)
            nc.vector.tensor_copy(out=lo[:], in_=res[:])

            # ---- row norms ----
            sq = work.tile([128, C], F32)
            qss0 = work.tile([128, 1], F32)
            qss1 = work.tile([64, 1], F32)
            kss = work.tile([NK, 1], F32)
            nc.scalar.activation(out=sq[:], in_=q0[:], func=AF.Square,
                                 accum_out=qss0[:])
            nc.scalar.activation(out=sq[:64, :], in_=q1[:], func=AF.Square,
                                 accum_out=qss1[:])
            nc.scalar.activation(out=sq[:NK, :], in_=kt[:], func=AF.Square,
                                 accum_out=kss[:])
            qinv0 = work.tile([128, 1], F32)
            qinv1 = work.tile([64, 1], F32)
            kinv = work.tile([NK, 1], F32)
            nc.scalar.sqrt(qss0[:], qss0[:])
            nc.scalar.sqrt(qss1[:], qss1[:])
            nc.scalar.sqrt(kss[:], kss[:])
            nc.vector.reciprocal(out=qinv0[:], in_=qss0[:])
            nc.vector.reciprocal(out=qinv1[:], in_=qss1[:])
            nc.vector.reciprocal(out=kinv[:], in_=kss[:])

            khat = work.tile([NK, C], F32)
            nc.vector.tensor_scalar_mul(khat[:], kt[:], kinv[:])

            ix0 = work.tile([128, 8], U32)
            ix1 = work.tile([64, 8], U32)
            SelT = {}

            with (
                tc.tile_pool(name="pfront", bufs=4, space="PSUM") as pfp,
                tc.tile_pool(name="pcos", bufs=1, space="PSUM") as pcp,
            ):
                # ---- transposes (PE): contraction dim onto partitions ----
                QT = [work.tile([128, ROWS], F32, name=f"qt{j}", tag=f"qt{j}")
                      for j in range(JT)]
                KT = [work.tile([128, NK], F32, name=f"ktt{j}", tag=f"ktt{j}")
                      for j in range(JT)]
                for j in range(JT):
                    js = slice(j * 128, (j + 1) * 128)
                    p0 = pfp.tile([128, 128], F32, tag="ptr")
                    nc.tensor.transpose(out=p0[:], in_=q0[:, js],
                                        identity=ident[:])
                    nc.vector.tensor_copy(out=QT[j][:, 0:128], in_=p0[:])
                    p1 = pfp.tile([128, 64], F32, tag="ptr")
                    nc.tensor.transpose(out=p1[:], in_=q1[:, js],
                                        identity=ident[:64, :64])
                    nc.vector.tensor_copy(out=QT[j][:, 128:192], in_=p1[:])
                    p2 = pfp.tile([128, NK], F32, tag="ptr")
                    nc.tensor.transpose(out=p2[:], in_=khat[:, js],
                                        identity=ident[:NK, :NK])
                    nc.vector.tensor_copy(out=KT[j][:], in_=p2[:])

                # ---- cosine scores (q unnormalized; scale-invariant topk) --
                pc0 = pcp.tile([128, POOL], F32)
                pc1 = pcp.tile([64, POOL], F32)
                for l in range(2):
                    for j in range(JT):
                        nc.tensor.matmul(
                            out=pc0[l * 64:(l + 1) * 64, :],
                            lhsT=QT[j][:, l * 64:(l + 1) * 64],
                            rhs=KT[j][:, l * 30:(l + 1) * 30],
                            start=(j == 0), stop=(j == JT - 1),
                        )
                for j in range(JT):
                    nc.tensor.matmul(
                        out=pc1[:, :],
                        lhsT=QT[j][:, 128:192],
                        rhs=KT[j][:, 60:90],
                        start=(j == 0), stop=(j == JT - 1),
                    )

                cos0 = work.tile([128, POOL], F32)
                cos1 = work.tile([64, POOL], F32)
                nc.vector.tensor_copy(out=cos0[:], in_=pc0[:])
                nc.vector.tensor_copy(out=cos1[:], in_=pc1[:])

                # ---- top-5 per row ----
                v0 = work.tile([128, 8], F32)
                v1 = work.tile([64, 8], F32)
                nc.vector.max(out=v0[:], in_=cos0[:])
                nc.vector.max_index(out=ix0[:], in_max=v0[:], in_values=cos0[:])
                nc.vector.max(out=v1[:], in_=cos1[:])
                nc.vector.max_index(out=ix1[:], in_max=v1[:], in_values=cos1[:])

                # ---- loss partial ----
                vs0 = work.tile([128, 1], F32)
                vs1 = work.tile([64, 1], F32)
                nc.vector.reduce_sum(out=vs0[:], in_=v0[:, 0:TOPK],
                                     axis=mybir.AxisListType.X)
                nc.vector.reduce_sum(out=vs1[:], in_=v1[:, 0:TOPK],
                                     axis=mybir.AxisListType.X)
                nc.vector.tensor_tensor(out=vs0[:], in0=vs0[:], in1=qinv0[:],
                                        op=mybir.AluOpType.mult)
                nc.vector.tensor_tensor(out=vs1[:], in0=vs1[:], in1=qinv1[:],
                                        op=mybir.AluOpType.mult)
                pl = pcp.tile([1, 1], F32)
                nc.tensor.matmul(out=pl[:], lhsT=vs0[:], rhs=ones[:128, :],
                                 start=True, stop=False, skip_group_check=True)
                nc.tensor.matmul(out=pl[:], lhsT=vs1[:], rhs=ones[:64, :],
                                 start=False, stop=True, skip_group_check=True)
                ls = work.tile([1, 1], F32)
                nc.vector.tensor_copy(out=ls[:], in_=pl[:])
                nc.sync.dma_start(out=lsum[:, :], in_=ls[:])

                # ---- global block index: idx + 30*l ----
                nc.vector.tensor_scalar_add(out=ix0[64:128, 0:TOPK],
                                            in0=ix0[64:128, 0:TOPK], scalar1=30)
                nc.vector.tensor_scalar_add(out=ix1[:, 0:TOPK],
                                            in0=ix1[:, 0:TOPK], scalar1=60)

                # ---- selection matrices for the PE-gather chunks ----
                # SelT[c][k, m] = (idx[block m of chunk c] == k), bf16 one-hot
                for (i, rg) in chunks:
                    if (i, rg) in dma_chunks:
                        continue
                    ixt, rows = (ix0, 128) if rg == 0 else (ix1, 64)
                    ixf = work.tile([rows, 1], F32, name=f"ixf_{i}_{rg}",
                                    tag="ixf")
                    nc.vector.tensor_copy(out=ixf[:], in_=ixt[:, i:i + 1])
                    pT = pfp.tile([128, rows], F32, tag="ptr",
                                  name=f"pT_{i}_{rg}")
                    nc.tensor.transpose(
                        out=pT[:],
                        in_=ixf[:].to_broadcast([rows, 128]),
                        identity=ident[:rows, :rows],
                    )
                    st = selp.tile([NK, 128], BF16, name=f"sel_{i}_{rg}",
                                   tag=f"sel_{i}_{rg}")
                    nc.vector.tensor_scalar(
                        out=st[:, :rows], in0=pT[0:NK, :rows],
                        scalar1=iota_f[:NK, :], scalar2=None,
                        op0=mybir.AluOpType.is_equal,
                    )
                    SelT[(i, rg)] = st

            # ---- gather + write out ----
            pout_r = pout[:, :].rearrange("(r i) e -> r i e", i=TOPK)
            with tc.tile_pool(name="pg", bufs=7, space="PSUM") as pgp:
                for ci, (i, rg) in enumerate(chunks):
                    ixt, rows = (ix0, 128) if rg == 0 else (ix1, 64)
                    st = stage.tile([rows, BLK], F32, tag=f"st{rg}",
                                    name=f"st_{i}_{rg}")
                    if (i, rg) in dma_chunks:
                        nc.gpsimd.indirect_dma_start(
                            out=st[:],
                            out_offset=None,
                            in_=ep[:, :],
                            in_offset=bass.IndirectOffsetOnAxis(
                                ap=ixt[:, i:i + 1], axis=0
                            ),
                        )
                    else:
                        sel = SelT[(i, rg)]
                        for nb in range(NB):
                            n0 = nb * 512
                            n = min(512, BLK - n0)
                            pb = pgp.tile([128, 512], F32, tag="gb",
                                          name=f"gb_{i}_{rg}_{nb}")
                            for p, plane in enumerate((hi, md, lo)):
                                nc.tensor.matmul(
                                    out=pb[:rows, :n],
                                    lhsT=sel[:, :rows],
                                    rhs=plane[:, n0:n0 + n],
                                    start=(p == 0), stop=(p == 2),
                                )
                            # evac PSUM -> SBUF staging (DVE + ACT split)
                            cp = nc.vector.tensor_copy if nb % 4 != 3 \
                                else nc.scalar.copy
                            cp(out=st[:, n0:n0 + n], in_=pb[:rows, :n])
                    nc.sync.dma_start(
                        out=pout_r[rg * 128:rg * 128 + rows, i, :],
                        in_=st[:],
                    )
    nc.compile()
    return nc


def kernel(x_query, e_p, e_k):
    x_query = np.ascontiguousarray(x_query, dtype=np.float32)
    e_p = np.ascontiguousarray(e_p, dtype=np.float32)
    e_k = np.ascontiguousarray(e_k, dtype=np.float32)

    in_maps = []
    for c in range(NCORES):
        lg, bh = divmod(c, 2)
        L0, b0 = 3 * lg, 64 * bh
        xs = np.transpose(x_query[b0:b0 + BSH, L0:L0 + LG, :], (1, 0, 2))
        in_maps.append({
            "xq": np.ascontiguousarray(xs.reshape(ROWS, C)),
            "ek": np.ascontiguousarray(e_k[L0:L0 + LG].reshape(NK, C)),
            "ep": np.ascontiguousarray(e_p[L0:L0 + LG].reshape(NK, BLK)),
        })

    nc = _build()
    res = run_bass_kernel_spmd(
        nc, in_maps, core_ids=list(range(NCORES)),
        trace=bool(int(os.environ.get("KERNEL_TRACE", "0"))),
        tmpdir=os.environ.get("KERNEL_TMPDIR") or None,
    )
    global LAST_EXEC_NS
    LAST_EXEC_NS = res.exec_time_ns

    p_return = np.empty((NL, B, TOPK * PLEN, EMB), np.float32)
    total = 0.0
    for c, r in enumerate(res.results):
        lg, bh = divmod(c, 2)
        L0, b0 = 3 * lg, 64 * bh
        p_return[L0:L0 + LG, b0:b0 + BSH] = r["pout"].reshape(
            LG, BSH, TOPK * PLEN, EMB
        )
        total += float(r["lsum"][0, 0])
    loss = np.float32(NL * TOPK - total / B)
    return p_return, loss


# revision 9
# speedup vs baseline: 1.1783x; 1.1783x over previous
"""Trainium2 Bass kernel for the topk_masking L2P problem.

Computation (matches the reference nn.Module forward):
  q = l2norm(x_query); nK = l2norm(e_k)
  cos = einsum('blj,lkj->blk', q, nK)           # [B, nL, pool]
  idx = top_k(cos, 5).indices                   # [B, nL, 5]
  P_  = e_p[l, idx]                             # gather -> [nL, B, 25, emb]
  loss = sum(1 - topk_values) / B               # == nL*5 - sum(topk_vals)/B

Sharding over 8 cores: 4 layer-groups (3 layers each) x 2 batch-halves (64).

The dominant cost is materializing the gathered output (14.75MB/core HBM
write).  To avoid also re-reading e_p blocks from HBM (another 14.75MB),
most gather chunks are computed ON-CHIP as one-hot matmuls on the tensor
engine: e_p is decomposed once into three bf16 planes (hi/mid/lo, whose
f32 sum reconstructs e_p bit-exactly) and each output chunk is
Sel^T @ plane accumulated over the three planes in PSUM (exact), then
evacuated to SBUF and streamed out with large DMAs.  A few chunks go
through the SWDGE indirect-DMA gather instead to balance PE vs DMA time.
"""

import os

import numpy as np

import concourse.bacc as bacc
import concourse.bass as bass
import concourse.tile as tile
from concourse import mybir
from concourse.bass_utils import run_bass_kernel_spmd
from concourse.masks import make_identity

F32 = mybir.dt.float32
BF16 = mybir.dt.bfloat16
U32 = mybir.dt.uint32
I32 = mybir.dt.int32

B, NL, C = 128, 12, 768
POOL, PLEN, EMB = 30, 5, 768
TOPK = 5
NCORES = 8
LG = 3            # layers per core
BSH = 64          # batches per core
ROWS = LG * BSH   # 192 rows of (layer, batch)
NK = LG * POOL    # 90 keys per core
BLK = PLEN * EMB  # 3840 f32 per prompt block
JT = C // 128     # 6 contraction chunks
NB = 8            # psum-bank chunks per block-chunk (7x512 + 1x256)

LAST_EXEC_NS = None

# chunks (i, rg) routed through the DMA indirect gather instead of PE
N_DMA_CHUNKS = int(os.environ.get("KERNEL_DMA_CHUNKS", "2"))


def _build():
    nc = bacc.Bacc(
        "TRN2", target_bir_lowering=False, debug=False, num_devices=NCORES
    )
    xq = nc.dram_tensor("xq", [ROWS, C], F32, kind="ExternalInput")
    ek = nc.dram_tensor("ek", [NK, C], F32, kind="ExternalInput")
    ep = nc.dram_tensor("ep", [NK, BLK], F32, kind="ExternalInput")
    pout = nc.dram_tensor("pout", [ROWS * TOPK, BLK], F32, kind="ExternalOutput")
    lsum = nc.dram_tensor("lsum", [1, 1], F32, kind="ExternalOutput")

    AF = mybir.ActivationFunctionType

    # chunk list: (i, rowgroup) -> 128/64 output blocks each
    chunks = [(i, rg) for i in range(TOPK) for rg in (0, 1)]
    dma_chunks = set(chunks[:N_DMA_CHUNKS])

    with tile.TileContext(nc) as tc:
        with (
            tc.tile_pool(name="const", bufs=1) as constp,
            tc.tile_pool(name="work", bufs=1) as work,
            tc.tile_pool(name="planes", bufs=1) as planes,
            tc.tile_pool(name="selp", bufs=1) as selp,
            tc.tile_pool(name="stage", bufs=3) as stage,
        ):
            ident = constp.tile([128, 128], F32)
            make_identity(nc, ident[:])
            ones = constp.tile([128, 1], F32)
            nc.vector.memset(ones[:], 1.0)
            iota_i = constp.tile([128, 1], I32)
            nc.gpsimd.iota(iota_i[:], pattern=[[0, 1]], base=0,
                           channel_multiplier=1)
            iota_f = constp.tile([128, 1], F32)
            nc.vector.tensor_copy(out=iota_f[:], in_=iota_i[:])

            # ---- load inputs (kt first: it heads the critical path) ----
            q0 = work.tile([128, C], F32)   # rows (l,b): l*64+b for l in {0,1}
            q1 = work.tile([64, C], F32)    # l = 2
            kt = work.tile([NK, C], F32)    # rows l*30+k
            nc.sync.dma_start(out=kt[:], in_=ek[:, :])
            nc.scalar.dma_start(out=q0[:], in_=xq[0:128, :])
            nc.scalar.dma_start(out=q1[:], in_=xq[128:192, :])

            # ---- e_p -> bf16 planes (hi+mid+lo sums back to f32 exactly) ----
            epf = work.tile([NK, BLK], F32)
            nc.sync.dma_start(out=epf[:], in_=ep[:, :])
            hi = planes.tile([NK, BLK], BF16)
            md = planes.tile([NK, BLK], BF16)
            lo = planes.tile([NK, BLK], BF16)
            res = work.tile([NK, BLK], F32)
            nc.vector.tensor_copy(out=hi[:], in_=epf[:])
            nc.vector.tensor_tensor(out=res[:], in0=epf[:], in1=hi[:],
                                    op=mybir.AluOpType.subtract)
            nc.vector.tensor_copy(out=md[:], in_=res[:])
            nc.vector.tensor_tensor(out=res[:], in0=res[:], in1=md[:],
                                    op=mybir.AluOpType.subtract)
            nc.vector.tensor_copy(out=lo[:], in_=res[:])

            # ---- row norms ----
            sq = work.tile([128, C], F32)
            qss0 = work.tile([128, 1], F32)
            qss1 = work.tile([64, 1], F32)
            kss = work.tile([NK, 1], F32)
            nc.scalar.activation(out=sq[:], in_=q0[:], func=AF.Square,
                                 accum_out=qss0[:])
            nc.scalar.activation(out=sq[:64, :], in_=q1[:], func=AF.Square,
                                 accum_out=qss1[:])
            nc.scalar.activation(out=sq[:NK, :], in_=kt[:], func=AF.Square,
                                 accum_out=kss[:])
            qinv0 = work.tile([128, 1], F32)
            qinv1 = work.tile([64, 1], F32)
            kinv = work.tile([NK, 1], F32)
            nc.scalar.sqrt(qss0[:], qss0[:])
            nc.scalar.sqrt(qss1[:], qss1[:])
            nc.scalar.sqrt(kss[:], kss[:])
            nc.vector.reciprocal(out=qinv0[:], in_=qss0[:])
            nc.vector.reciprocal(out=qinv1[:], in_=qss1[:])
            nc.vector.reciprocal(out=kinv[:], in_=kss[:])

            ix0 = work.tile([128, 8], U32)
            ix1 = work.tile([64, 8], U32)
            SelT = {}

            with (
                tc.tile_pool(name="pfront", bufs=4, space="PSUM") as pfp,
                tc.tile_pool(name="pcos", bufs=1, space="PSUM") as pcp,
            ):
                # ---- transposes (PE): contraction dim onto partitions ----
                QT = [work.tile([128, ROWS], F32, name=f"qt{j}", tag=f"qt{j}")
                      for j in range(JT)]
                KT = [work.tile([128, NK], F32, name=f"ktt{j}", tag=f"ktt{j}")
                      for j in range(JT)]
                for j in range(JT):
                    js = slice(j * 128, (j + 1) * 128)
                    p0 = pfp.tile([128, 128], F32, tag="ptr")
                    nc.tensor.transpose(out=p0[:], in_=q0[:, js],
                                        identity=ident[:])
                    nc.vector.tensor_copy(out=QT[j][:, 0:128], in_=p0[:])
                    p1 = pfp.tile([128, 64], F32, tag="ptr")
                    nc.tensor.transpose(out=p1[:], in_=q1[:, js],
                                        identity=ident[:64, :64])
                    nc.vector.tensor_copy(out=QT[j][:, 128:192], in_=p1[:])
                    p2 = pfp.tile([128, NK], F32, tag="ptr")
                    nc.tensor.transpose(out=p2[:], in_=khat[:, js],
                                        identity=ident[:NK, :NK])
                    nc.vector.tensor_copy(out=KT[j][:], in_=p2[:])

                # ---- cosine scores (q unnormalized; scale-invariant topk) --
                pc0 = pcp.tile([128, POOL], F32)
                pc1 = pcp.tile([64, POOL], F32)
                for l in range(2):
                    for j in range(JT):
                        nc.tensor.matmul(
                            out=pc0[l * 64:(l + 1) * 64, :],
                            lhsT=QT[j][:, l * 64:(l + 1) * 64],
                            rhs=KT[j][:, l * 30:(l + 1) * 30],
                            start=(j == 0), stop=(j == JT - 1),
                        )
                for j in range(JT):
                    nc.tensor.matmul(
                        out=pc1[:, :],
                        lhsT=QT[j][:, 128:192],
                        rhs=KT[j][:, 60:90],
                        start=(j == 0), stop=(j == JT - 1),
                    )

                cos0 = work.tile([128, POOL], F32)
                cos1 = work.tile([64, POOL], F32)
                nc.vector.tensor_copy(out=cos0[:], in_=pc0[:])
                nc.vector.tensor_copy(out=cos1[:], in_=pc1[:])

                # ---- top-5 per row ----
                v0 = work.tile([128, 8], F32)
                v1 = work.tile([64, 8], F32)
                nc.vector.max(out=v0[:], in_=cos0[:])
                nc.vector.max_index(out=ix0[:], in_max=v0[:], in_values=cos0[:])
                nc.vector.max(out=v1[:], in_=cos1[:])
                nc.vector.max_index(out=ix1[:], in_max=v1[:], in_values=cos1[:])

                # ---- loss partial ----
                vs0 = work.tile([128, 1], F32)
                vs1 = work.tile([64, 1], F32)
                nc.vector.reduce_sum(out=vs0[:], in_=v0[:, 0:TOPK],
                                     axis=mybir.AxisListType.X)
                nc.vector.reduce_sum(out=vs1[:], in_=v1[:, 0:TOPK],
                                     axis=mybir.AxisListType.X)
                nc.vector.tensor_tensor(out=vs0[:], in0=vs0[:], in1=qinv0[:],
                                        op=mybir.AluOpType.mult)
                nc.vector.tensor_tensor(out=vs1[:], in0=vs1[:], in1=qinv1[:],
                                        op=mybir.AluOpType.mult)
                pl = pcp.tile([1, 1], F32)
                nc.tensor.matmul(out=pl[:], lhsT=vs0[:], rhs=ones[:128, :],
                                 start=True, stop=False, skip_group_check=True)
                nc.tensor.matmul(out=pl[:], lhsT=vs1[:], rhs=ones[:64, :],
                                 start=False, stop=True, skip_group_check=True)
                ls = work.tile([1, 1], F32)
                nc.vector.tensor_copy(out=ls[:], in_=pl[:])
                nc.sync.dma_start(out=lsum[:, :], in_=ls[:])

                # ---- global block index: idx + 30*l ----
                nc.vector.tensor_scalar_add(out=ix0[64:128, 0:TOPK],
                                            in0=ix0[64:128, 0:TOPK], scalar1=30)
                nc.vector.tensor_scalar_add(out=ix1[:, 0:TOPK],
                                            in0=ix1[:, 0:TOPK], scalar1=60)

                # ---- selection matrices for the PE-gather chunks ----
                # SelT[c][k, m] = (idx[block m of chunk c] == k), bf16 one-hot
                for (i, rg) in chunks:
                    if (i, rg) in dma_chunks:
                        continue
                    ixt, rows = (ix0, 128) if rg == 0 else (ix1, 64)
                    ixf = work.tile([rows, 1], F32, name=f"ixf_{i}_{rg}",
                                    tag="ixf")
                    nc.vector.tensor_copy(out=ixf[:], in_=ixt[:, i:i + 1])
                    pT = pfp.tile([128, rows], F32, tag="ptr",
                                  name=f"pT_{i}_{rg}")
                    nc.tensor.transpose(
                        out=pT[:],
                        in_=ixf[:].to_broadcast([rows, 128]),
                        identity=ident[:rows, :rows],
                    )
                    st = selp.tile([NK, 128], BF16, name=f"sel_{i}_{rg}",
                                   tag=f"sel_{i}_{rg}")
                    nc.vector.tensor_scalar(
                        out=st[:, :rows], in0=pT[0:NK, :rows],
                        scalar1=iota_f[:NK, :], scalar2=None,
                        op0=mybir.AluOpType.is_equal,
                    )
                    SelT[(i, rg)] = st

            # ---- gather + write out ----
            pout_r = pout[:, :].rearrange("(r i) e -> r i e", i=TOPK)
            with tc.tile_pool(name="pg", bufs=7, space="PSUM") as pgp:
                for ci, (i, rg) in enumerate(chunks):
                    ixt, rows = (ix0, 128) if rg == 0 else (ix1, 64)
                    st = stage.tile([rows, BLK], F32, tag=f"st{rg}",
                                    name=f"st_{i}_{rg}")
                    if (i, rg) in dma_chunks:
                        nc.gpsimd.indirect_dma_start(
                            out=st[:],
                            out_offset=None,
                            in_=ep[:, :],
                            in_offset=bass.IndirectOffsetOnAxis(
                                ap=ixt[:, i:i + 1], axis=0
                            ),
                        )
                    else:
                        sel = SelT[(i, rg)]
                        for nb in range(NB):
                            n0 = nb * 512
                            n = min(512, BLK - n0)
                            pb = pgp.tile([128, 512], F32, tag="gb",
                                          name=f"gb_{i}_{rg}_{nb}")
                            for p, plane in enumerate((hi, md, lo)):
                                nc.tensor.matmul(
                                    out=pb[:rows, :n],
                                    lhsT=sel[:, :rows],
                                    rhs=plane[:, n0:n0 + n],
                                    start=(p == 0), stop=(p == 2),
                                )
                            # evac PSUM -> SBUF staging (DVE + ACT split)
                            cp = nc.vector.tensor_copy if nb % 4 != 3 \
                                else nc.scalar.copy
                            cp(out=st[:, n0:n0 + n], in_=pb[:rows, :n])
                    nc.sync.dma_start(
                        out=pout_r[rg * 128:rg * 128 + rows, i, :],
                        in_=st[:],
                    )
    nc.compile()
    return nc


def kernel(x_query, e_p, e_k):
    x_query = np.ascontiguousarray(x_query, dtype=np.float32)
    e_p = np.ascontiguousarray(e_p, dtype=np.float32)
    e_k = np.ascontiguousarray(e_k, dtype=np.float32)

    in_maps = []
    for c in range(NCORES):
        lg, bh = divmod(c, 2)
        L0, b0 = 3 * lg, 64 * bh
        xs = np.transpose(x_query[b0:b0 + BSH, L0:L0 + LG, :], (1, 0, 2))
        in_maps.append({
            "xq": np.ascontiguousarray(xs.reshape(ROWS, C)),
            "ek": np.ascontiguousarray(e_k[L0:L0 + LG].reshape(NK, C)),
            "ep": np.ascontiguousarray(e_p[L0:L0 + LG].reshape(NK, BLK)),
        })

    nc = _build()
    res = run_bass_kernel_spmd(
        nc, in_maps, core_ids=list(range(NCORES)),
        trace=bool(int(os.environ.get("KERNEL_TRACE", "0"))),
        tmpdir=os.environ.get("KERNEL_TMPDIR") or None,
    )
    global LAST_EXEC_NS
    LAST_EXEC_NS = res.exec_time_ns

    p_return = np.empty((NL, B, TOPK * PLEN, EMB), np.float32)
    total = 0.0
    for c, r in enumerate(res.results):
        lg, bh = divmod(c, 2)
        L0, b0 = 3 * lg, 64 * bh
        p_return[L0:L0 + LG, b0:b0 + BSH] = r["pout"].reshape(
            LG, BSH, TOPK * PLEN, EMB
        )
        total += float(r["lsum"][0, 0])
    loss = np.float32(NL * TOPK - total / B)
    return p_return, loss


# revision 14
# speedup vs baseline: 1.4100x; 1.1967x over previous
"""Trainium2 Bass kernel for the topk_masking L2P problem.

Computation (matches the reference nn.Module forward):
  q = l2norm(x_query); nK = l2norm(e_k)
  cos = einsum('blj,lkj->blk', q, nK)           # [B, nL, pool]
  idx = top_k(cos, 5).indices                   # [B, nL, 5]
  P_  = e_p[l, idx]                             # gather -> [nL, B, 25, emb]
  loss = sum(1 - topk_values) / B               # == nL*5 - sum(topk_vals)/B

Sharding over 8 cores: 4 layer-groups (3 layers each) x 2 batch-halves (64).

The dominant cost is materializing the gathered output (14.75MB/core HBM
write).  To avoid also re-reading e_p blocks from HBM (another 14.75MB),
most gather chunks are computed ON-CHIP as one-hot matmuls on the tensor
engine: e_p is decomposed once into three bf16 planes (hi/mid/lo, whose
f32 sum reconstructs e_p bit-exactly) and each output chunk is
Sel^T @ plane accumulated over the three planes in PSUM (exact), then
evacuated to SBUF and streamed out with large DMAs.  A few chunks go
through the SWDGE indirect-DMA gather instead to balance PE vs DMA time.
"""

import os

import numpy as np

import concourse.bacc as bacc
import concourse.bass as bass
import concourse.tile as tile
from concourse import mybir
from concourse.bass_utils import run_bass_kernel_spmd
from concourse.masks import make_identity

F32 = mybir.dt.float32
BF16 = mybir.dt.bfloat16
U32 = mybir.dt.uint32
I32 = mybir.dt.int32

B, NL, C = 128, 12, 768
POOL, PLEN, EMB = 30, 5, 768
TOPK = 5
NCORES = 8
LG = 3            # layers per core
BSH = 64          # batches per core
ROWS = LG * BSH   # 192 rows of (layer, batch)
NK = LG * POOL    # 90 keys per core
BLK = PLEN * EMB  # 3840 f32 per prompt block
JT = C // 128     # 6 contraction chunks
NB = 8            # psum-bank chunks per block-chunk (7x512 + 1x256)

LAST_EXEC_NS = None

# chunks (i, rg) routed through the DMA indirect gather instead of PE
N_DMA_CHUNKS = int(os.environ.get("KERNEL_DMA_CHUNKS", "5"))


def _build():
    nc = bacc.Bacc(
        "TRN2", target_bir_lowering=False, debug=False, num_devices=NCORES
    )
    xq = nc.dram_tensor("xq", [ROWS, C], F32, kind="ExternalInput")
    ek = nc.dram_tensor("ek", [NK, C], F32, kind="ExternalInput")
    ep = nc.dram_tensor("ep", [NK, BLK], F32, kind="ExternalInput")
    pout = nc.dram_tensor("pout", [ROWS * TOPK, BLK], F32, kind="ExternalOutput")
    lsum = nc.dram_tensor("lsum", [1, 1], F32, kind="ExternalOutput")

    AF = mybir.ActivationFunctionType

    # chunk list: (i, rowgroup) -> 128/64 output blocks each
    chunks = [(i, rg) for i in range(TOPK) for rg in (0, 1)]
    dma_chunks = set(chunks[:N_DMA_CHUNKS])

    with tile.TileContext(nc) as tc:
        with (
            tc.tile_pool(name="const", bufs=1) as constp,
            tc.tile_pool(name="work", bufs=1) as work,
            tc.tile_pool(name="planes", bufs=1) as planes,
            tc.tile_pool(name="selp", bufs=1) as selp,
            tc.tile_pool(name="stage", bufs=3) as stage,
        ):
            ident = constp.tile([128, 128], F32)
            make_identity(nc, ident[:])
            ones = constp.tile([128, 1], F32)
            nc.vector.memset(ones[:], 1.0)
            iota_i = constp.tile([128, 1], I32)
            nc.gpsimd.iota(iota_i[:], pattern=[[0, 1]], base=0,
                           channel_multiplier=1)
            iota_f = constp.tile([128, 1], F32)
            nc.vector.tensor_copy(out=iota_f[:], in_=iota_i[:])

            # ---- load inputs (kt first: it heads the critical path) ----
            q0 = work.tile([128, C], F32)   # rows (l,b): l*64+b for l in {0,1}
            q1 = work.tile([64, C], F32)    # l = 2
            kt = work.tile([NK, C], F32)    # rows l*30+k
            nc.sync.dma_start(out=kt[:], in_=ek[:, :])
            nc.scalar.dma_start(out=q0[:], in_=xq[0:128, :])
            nc.scalar.dma_start(out=q1[:], in_=xq[128:192, :])

            # ---- e_p load (decomposition emitted later, off the idx path) --
            epf = work.tile([NK, BLK], F32)
            nc.sync.dma_start(out=epf[:], in_=ep[:, :])
            hi = planes.tile([NK, BLK], BF16)
            md = planes.tile([NK, BLK], BF16)
            lo = planes.tile([NK, BLK], BF16)

            # ---- row norms ----
            sq = work.tile([128, C], F32)
            qss0 = work.tile([128, 1], F32)
            qss1 = work.tile([64, 1], F32)
            kss = work.tile([NK, 1], F32)
            nc.scalar.activation(out=sq[:], in_=q0[:], func=AF.Square,
                                 accum_out=qss0[:])
            nc.scalar.activation(out=sq[:64, :], in_=q1[:], func=AF.Square,
                                 accum_out=qss1[:])
            nc.scalar.activation(out=sq[:NK, :], in_=kt[:], func=AF.Square,
                                 accum_out=kss[:])
            qinv0 = work.tile([128, 1], F32)
            qinv1 = work.tile([64, 1], F32)
            kinv = work.tile([NK, 1], F32)
            nc.scalar.sqrt(qss0[:], qss0[:])
            nc.scalar.sqrt(qss1[:], qss1[:])
            nc.scalar.sqrt(kss[:], kss[:])
            nc.vector.reciprocal(out=qinv0[:], in_=qss0[:])
            nc.vector.reciprocal(out=qinv1[:], in_=qss1[:])
            nc.vector.reciprocal(out=kinv[:], in_=kss[:])

            ix0 = work.tile([128, 8], U32)
            ix1 = work.tile([64, 8], U32)
            SelT = {}

            with (
                tc.tile_pool(name="pfront", bufs=4, space="PSUM") as pfp,
                tc.tile_pool(name="pcos", bufs=1, space="PSUM") as pcp,
            ):
                # ---- transposes (PE): contraction dim onto partitions ----
                QT = [work.tile([128, ROWS], F32, name=f"qt{j}", tag=f"qt{j}")
                      for j in range(JT)]
                KT = [work.tile([128, NK], F32, name=f"ktt{j}", tag=f"ktt{j}")
                      for j in range(JT)]
                for j in range(JT):
                    js = slice(j * 128, (j + 1) * 128)
                    p0 = pfp.tile([128, 128], F32, tag="ptr")
                    nc.tensor.transpose(out=p0[:], in_=q0[:, js],
                                        identity=ident[:])
                    nc.vector.tensor_copy(out=QT[j][:, 0:128], in_=p0[:])
                    p1 = pfp.tile([128, 64], F32, tag="ptr")
                    nc.tensor.transpose(out=p1[:], in_=q1[:, js],
                                        identity=ident[:64, :64])
                    nc.vector.tensor_copy(out=QT[j][:, 128:192], in_=p1[:])
                    p2 = pfp.tile([128, NK], F32, tag="ptr")
                    nc.tensor.transpose(out=p2[:], in_=kt[:, js],
                                        identity=ident[:NK, :NK])
                    nc.vector.tensor_copy(out=KT[j][:], in_=p2[:])

                # ---- cosine scores (q unnormalized; scale-invariant topk) --
                pc0 = pcp.tile([128, POOL], F32)
                pc1 = pcp.tile([64, POOL], F32)
                for l in range(2):
                    for j in range(JT):
                        nc.tensor.matmul(
                            out=pc0[l * 64:(l + 1) * 64, :],
                            lhsT=QT[j][:, l * 64:(l + 1) * 64],
                            rhs=KT[j][:, l * 30:(l + 1) * 30],
                            start=(j == 0), stop=(j == JT - 1),
                        )
                for j in range(JT):
                    nc.tensor.matmul(
                        out=pc1[:, :],
                        lhsT=QT[j][:, 128:192],
                        rhs=KT[j][:, 60:90],
                        start=(j == 0), stop=(j == JT - 1),
                    )

                # broadcast kinv across partitions via PE: kinvB[m, n] = kinv[n]
                pkb = pfp.tile([128, NK], F32, tag="ptr")
                nc.tensor.transpose(out=pkb[:], in_=kinv[:].to_broadcast([NK, 128]),
                                    identity=ident[:NK, :NK])
                kinvB = work.tile([128, NK], F32)
                nc.vector.tensor_copy(out=kinvB[:], in_=pkb[:])

                # cos = (q . k) * kinv[key]  (key-normalized; q scale folded
                # into the topk values later — topk order is scale-invariant)
                cos0 = work.tile([128, POOL], F32)
                cos1 = work.tile([64, POOL], F32)
                nc.vector.tensor_tensor(out=cos0[0:64, :], in0=pc0[0:64, :],
                                        in1=kinvB[0:64, 0:30],
                                        op=mybir.AluOpType.mult)
                nc.vector.tensor_tensor(out=cos0[64:128, :], in0=pc0[64:128, :],
                                        in1=kinvB[64:128, 30:60],
                                        op=mybir.AluOpType.mult)
                nc.vector.tensor_tensor(out=cos1[:, :], in0=pc1[:, :],
                                        in1=kinvB[0:64, 60:90],
                                        op=mybir.AluOpType.mult)

                # ---- top-5 per row ----
                v0 = work.tile([128, 8], F32)
                v1 = work.tile([64, 8], F32)
                nc.vector.max(out=v0[:], in_=cos0[:])
                nc.vector.max_index(out=ix0[:], in_max=v0[:], in_values=cos0[:])
                nc.vector.max(out=v1[:], in_=cos1[:])
                nc.vector.max_index(out=ix1[:], in_max=v1[:], in_values=cos1[:])

                # ---- loss partial ----
                vs0 = work.tile([128, 1], F32)
                vs1 = work.tile([64, 1], F32)
                nc.vector.reduce_sum(out=vs0[:], in_=v0[:, 0:TOPK],
                                     axis=mybir.AxisListType.X)
                nc.vector.reduce_sum(out=vs1[:], in_=v1[:, 0:TOPK],
                                     axis=mybir.AxisListType.X)
                nc.vector.tensor_tensor(out=vs0[:], in0=vs0[:], in1=qinv0[:],
                                        op=mybir.AluOpType.mult)
                nc.vector.tensor_tensor(out=vs1[:], in0=vs1[:], in1=qinv1[:],
                                        op=mybir.AluOpType.mult)
                pl = pcp.tile([1, 1], F32)
                nc.tensor.matmul(out=pl[:], lhsT=vs0[:], rhs=ones[:128, :],
                                 start=True, stop=False, skip_group_check=True)
                nc.tensor.matmul(out=pl[:], lhsT=vs1[:], rhs=ones[:64, :],
                                 start=False, stop=True, skip_group_check=True)
                ls = work.tile([1, 1], F32)
                nc.vector.tensor_copy(out=ls[:], in_=pl[:])
                nc.sync.dma_start(out=lsum[:, :], in_=ls[:])

                # ---- global block index: idx + 30*l ----
                nc.vector.tensor_scalar_add(out=ix0[64:128, 0:TOPK],
                                            in0=ix0[64:128, 0:TOPK], scalar1=30)
                nc.vector.tensor_scalar_add(out=ix1[:, 0:TOPK],
                                            in0=ix1[:, 0:TOPK], scalar1=60)

                # ---- e_p -> bf16 planes (hi+mid+lo == e_p bit-exactly) ----
                # casts on ACT, subtracts on DVE; emitted after the topk
                # chain so the scheduler keeps DVE free for the idx path.
                res = work.tile([NK, BLK], F32)
                nc.scalar.copy(out=hi[:], in_=epf[:])
                nc.vector.tensor_tensor(out=res[:], in0=epf[:], in1=hi[:],
                                        op=mybir.AluOpType.subtract)
                nc.scalar.copy(out=md[:], in_=res[:])
                nc.vector.tensor_tensor(out=res[:], in0=res[:], in1=md[:],
                                        op=mybir.AluOpType.subtract)
                nc.scalar.copy(out=lo[:], in_=res[:])

                # ---- selection matrices for the PE-gather chunks ----
                # SelT[c][k, m] = (idx[block m of chunk c] == k), bf16 one-hot
                for (i, rg) in chunks:
                    if (i, rg) in dma_chunks:
                        continue
                    ixt, rows = (ix0, 128) if rg == 0 else (ix1, 64)
                    ixf = work.tile([rows, 1], F32, name=f"ixf_{i}_{rg}",
                                    tag="ixf")
                    nc.vector.tensor_copy(out=ixf[:], in_=ixt[:, i:i + 1])
                    pT = pfp.tile([128, rows], F32, tag="ptr",
                                  name=f"pT_{i}_{rg}")
                    nc.tensor.transpose(
                        out=pT[:],
                        in_=ixf[:].to_broadcast([rows, 128]),
                        identity=ident[:rows, :rows],
                    )
                    st = selp.tile([NK, 128], BF16, name=f"sel_{i}_{rg}",
                                   tag=f"sel_{i}_{rg}")
                    nc.vector.tensor_scalar(
                        out=st[:, :rows], in0=pT[0:NK, :rows],
                        scalar1=iota_f[:NK, :], scalar2=None,
                        op0=mybir.AluOpType.is_equal,
                    )
                    SelT[(i, rg)] = st

            # ---- gather + write out ----
            pout_r = pout[:, :].rearrange("(r i) e -> r i e", i=TOPK)
            with tc.tile_pool(name="pg", bufs=7, space="PSUM") as pgp:
                for ci, (i, rg) in enumerate(chunks):
                    ixt, rows = (ix0, 128) if rg == 0 else (ix1, 64)
                    st = stage.tile([rows, BLK], F32, tag=f"st{rg}",
                                    name=f"st_{i}_{rg}")
                    if (i, rg) in dma_chunks:
                        nc.gpsimd.indirect_dma_start(
                            out=st[:],
                            out_offset=None,
                            in_=ep[:, :],
                            in_offset=bass.IndirectOffsetOnAxis(
                                ap=ixt[:, i:i + 1], axis=0
                            ),
                        )
                    else:
                        sel = SelT[(i, rg)]
                        for nb in range(NB):
                            n0 = nb * 512
                            n = min(512, BLK - n0)
                            pb = pgp.tile([128, 512], F32, tag="gb",
                                          name=f"gb_{i}_{rg}_{nb}")
                            for p, plane in enumerate((hi, md, lo)):
                                nc.tensor.matmul(
                                    out=pb[:rows, :n],
                                    lhsT=sel[:, :rows],
                                    rhs=plane[:, n0:n0 + n],
                                    start=(p == 0), stop=(p == 2),
                                )
                            # evac PSUM -> SBUF staging (DVE + ACT split)
                            cp = nc.vector.tensor_copy if nb % 4 != 3 \
                                else nc.scalar.copy
                            cp(out=st[:, n0:n0 + n], in_=pb[:rows, :n])
                    nc.sync.dma_start(
                        out=pout_r[rg * 128:rg * 128 + rows, i, :],
                        in_=st[:],
                    )
    nc.compile()
    return nc


def kernel(x_query, e_p, e_k):
    x_query = np.ascontiguousarray(x_query, dtype=np.float32)
    e_p = np.ascontiguousarray(e_p, dtype=np.float32)
    e_k = np.ascontiguousarray(e_k, dtype=np.float32)

    in_maps = []
    for c in range(NCORES):
        lg, bh = divmod(c, 2)
        L0, b0 = 3 * lg, 64 * bh
        xs = np.transpose(x_query[b0:b0 + BSH, L0:L0 + LG, :], (1, 0, 2))
        in_maps.append({
            "xq": np.ascontiguousarray(xs.reshape(ROWS, C)),
            "ek": np.ascontiguousarray(e_k[L0:L0 + LG].reshape(NK, C)),
            "ep": np.ascontiguousarray(e_p[L0:L0 + LG].reshape(NK, BLK)),
        })

    nc = _build()
    res = run_bass_kernel_spmd(
        nc, in_maps, core_ids=list(range(NCORES)),
        trace=bool(int(os.environ.get("KERNEL_TRACE", "0"))),
        tmpdir=os.environ.get("KERNEL_TMPDIR") or None,
    )
    global LAST_EXEC_NS
    LAST_EXEC_NS = res.exec_time_ns

    p_return = np.empty((NL, B, TOPK * PLEN, EMB), np.float32)
    total = 0.0
    for c, r in enumerate(res.results):
        lg, bh = divmod(c, 2)
        L0, b0 = 3 * lg, 64 * bh
        p_return[L0:L0 + LG, b0:b0 + BSH] = r["pout"].reshape(
            LG, BSH, TOPK * PLEN, EMB
        )
        total += float(r["lsum"][0, 0])
    loss = np.float32(NL * TOPK - total / B)
    return p_return, loss
